# revision 6
# baseline (speedup 1.0000x reference)
"""DirectedGCNConv on 8 Trainium2 NeuronCores.

Math (reference):
    h    = x @ W
    agg  = segment_sum(w_e * h[src_e], tgt_e)
    deg  = segment_sum(w_e, tgt_e);  dinv = rsqrt(max(deg, 1))
    out  = (h + agg) * dinv + bias

Since segment-sum commutes with the feature matmul and dinv is a
per-target row scale:
    out[t] = ((dinv[t] x[t] + sum_e w_e dinv[t] x[src_e]) @ W) + bias
so dinv folds into per-edge values computed on the host, and the device
gathers raw x rows per edge (dma_gather), scatter-sums them per
128-target tile via one-hot matmuls into PSUM, applies the 128x128
weight matmul, and adds bias.  No h precompute: the weight matmul is
applied AFTER aggregation, once per target tile, so nothing but the
gathered rows ever moves through HBM.

The self term dinv[t]*x[t] is not gathered: the host prepares a
dinv-scaled, node-ordered, transposed copy of x (xs), which streams in
with one sequential DMA per run and enters psum2 as a second
accumulating matmul against W.

Sharding: node-parallel by target.  The 100k nodes are packed into
784 tiles of 128, bin-packed so that, per tile, the incoming-edge count
from source-index bank b fits in kbs[b] K-tiles of 128 slots (banks
29184/29184/29184/12800 rows at 2/2/2/1 K-tiles = 7 K-chunks per tile,
896 gather slots for a mean real edge load of 816).  Each core owns 98
tiles and processes every edge targeting its nodes.  x and W are
replicated; no collectives are needed.

The gather uses the Q7 dma_gather instruction (int16 indices, hence
banks of < 32768 rows).  All matmul/gather data is bf16; accumulation
stays fp32 in PSUM.  One-hot scatter matrices are built in-loop with
one fused tensor_scalar (is_equal then mult with fp32 per-partition
scalars) per K-chunk on the vector engine.
"""

import sys

import numpy as np

for _p in ("/opt/trn_rl_repo", "/root/.axon_site/_ro/trn_rl_repo"):
    if _p not in sys.path:
        sys.path.append(_p)

N_NODES = 100000
N_EDGES = 640000
D = 128
N_CORES = 8

P = 128                      # partitions / tile node count
SENTINEL = 200.0             # tgtl value for phantom slots (bf16-exact)


def _default_cfg():
    return dict(
        n_nodes=N_NODES,
        n_cores=N_CORES,
        tiles_per_core=98,
        run_tiles=14,                  # tiles per gather "run"
        bank_sizes=(29184, 29184, 29184, 12800),   # int16-safe, sum=n_pad
        kbs=(2, 2, 2, 1),              # K-tiles per (tile, bank)
    )


def _derived(cfg):
    kbs = cfg["kbs"]
    bank_sizes = cfg["bank_sizes"]
    nbanks = len(kbs)
    caps = [kb * P for kb in kbs]
    k_tot = sum(kbs)
    koff = np.concatenate([[0], np.cumsum(kbs)[:-1]]).astype(int)
    bank_starts = np.concatenate([[0], np.cumsum(bank_sizes)]).astype(int)
    return nbanks, caps, k_tot, koff, bank_starts


def _bf16():
    import ml_dtypes
    return ml_dtypes.bfloat16


# ----------------------------------------------------------------------------
# Host-side preprocessing
# ----------------------------------------------------------------------------

def _pack_nodes(tgt, cfg):
    """Assign each node to a (tile, slot) so that per-(tile, bank)
    edge counts are <= caps[b].  Returns node_order [n_tiles, P] holding
    node ids (or >= n_nodes for phantom slots)."""
    nbanks, caps, _, _, bank_starts = _derived(cfg)
    n_tiles = cfg["n_cores"] * cfg["tiles_per_core"]
    n_slots = n_tiles * P
    caps = np.asarray(caps)

    src = cfg["_src"]
    bank_of_src = (np.searchsorted(bank_starts, src, "right") - 1).astype(
        np.int64)
    # per-node, per-bank demand (incoming edges by source bank)
    d = np.zeros((n_slots, nbanks), np.int64)
    np.add.at(d, (tgt, bank_of_src), 1)

    total = d.sum(1)
    order = np.argsort(-total, kind="stable")
    # serpentine init: exactly P nodes (incl. phantoms) per tile
    assign = np.empty(n_slots, np.int64)
    row = np.arange(n_slots) // n_tiles
    pos = np.arange(n_slots) % n_tiles
    assign[order] = np.where(row % 2 == 0, pos, n_tiles - 1 - pos)
    loads = np.zeros((n_tiles, nbanks), np.int64)
    np.add.at(loads, assign, d)
    counts = np.bincount(assign, minlength=n_tiles)
    assert (counts == P).all()

    # repair: swap nodes out of overloaded (tile, bank)
    members = [list(np.where(assign == t)[0]) for t in range(n_tiles)]
    for _ in range(400):
        over = np.argwhere(loads > caps[None, :])
        if len(over) == 0:
            break
        for t, b in over:
            while loads[t, b] > caps[b]:
                mem = members[t]
                n_out = mem[int(np.argmax(d[mem, b]))]
                # candidate destination tiles, lightest in bank b
                cand = np.argsort(loads[:, b] - caps[b])
                done = False
                for u in cand[:128]:
                    if u == t:
                        continue
                    memu = members[u]
                    n_in = memu[int(np.argmin(d[memu, b]))]
                    new_u = loads[u] - d[n_in] + d[n_out]
                    new_t = loads[t] - d[n_out] + d[n_in]
                    if (new_u <= caps).all() and (new_t[b] < loads[t, b]):
                        members[t].remove(n_out)
                        members[u].remove(n_in)
                        members[t].append(n_in)
                        members[u].append(n_out)
                        loads[t] = new_t
                        loads[u] = new_u
                        assign[n_out] = u
                        assign[n_in] = t
                        done = True
                        break
                if not done:
                    raise RuntimeError("packing repair failed")
    assert (loads <= caps[None, :]).all(), "node packing failed"

    node_order = np.empty((n_tiles, P), np.int64)
    for t in range(n_tiles):
        node_order[t] = sorted(members[t])
    return node_order


def preprocess(x, edge_index, edge_weight, cfg):
    """Build per-core device arrays.  Returns (per_core_inputs, node_order)."""
    bf16 = _bf16()
    nbanks, caps, k_tot, koff, bank_starts = _derived(cfg)
    n_nodes = cfg["n_nodes"]
    n_cores = cfg["n_cores"]
    tiles_per_core = cfg["tiles_per_core"]
    run_tiles = cfg["run_tiles"]
    n_tiles = n_cores * tiles_per_core
    runs = tiles_per_core // run_tiles
    call_slots = [run_tiles * caps[b] for b in range(nbanks)]
    # idx-plane column offset of each call (run-major, bank-minor), in
    # 16-element wraps
    call_cols = [cs // 16 for cs in call_slots]
    cols_per_run = sum(call_cols)
    k_cols = tiles_per_core * k_tot           # tgtl/v plane columns per core

    src = np.asarray(edge_index[0], np.int64)
    tgt = np.asarray(edge_index[1], np.int64)
    w = np.asarray(edge_weight, np.float32)
    cfg = dict(cfg)
    cfg["_src"] = src

    node_order = _pack_nodes(tgt, cfg)

    # degree normalization, folded into the per-edge values
    deg = np.zeros(n_nodes, np.float64)
    np.add.at(deg, tgt, w.astype(np.float64))
    dinv = (1.0 / np.sqrt(np.maximum(deg, 1.0))).astype(np.float32)

    # node -> (tile, local slot)
    tile_of = np.empty(n_tiles * P, np.int64)
    local_of = np.empty(n_tiles * P, np.int64)
    flat = node_order.reshape(-1)
    tile_of[flat] = np.arange(n_tiles * P) // P
    local_of[flat] = np.arange(n_tiles * P) % P

    # per-edge values: v = w_e * dinv[tgt]  (self term handled via xs)
    all_src = src
    all_tgt = tgt
    all_v = w * dinv[tgt]

    e_tile = tile_of[all_tgt]
    e_bank = (np.searchsorted(bank_starts, all_src, "right") - 1).astype(
        np.int64)
    order = np.lexsort((e_bank, e_tile))
    all_src, all_tgt, all_v, e_tile, e_bank = (
        a[order] for a in (all_src, all_tgt, all_v, e_tile, e_bank)
    )
    group_key = e_tile * nbanks + e_bank
    starts = np.searchsorted(group_key, np.arange(n_tiles * nbanks))
    ends = np.searchsorted(group_key, np.arange(n_tiles * nbanks) + 1)

    # dinv-scaled x^T in node order: xs[:, gt*P + p] = x[node].T * dinv[node]
    xf = np.asarray(x, np.float32)
    flat_valid = flat < n_nodes
    xs_all = np.zeros((D, n_tiles * P), np.float32)
    xs_all[:, flat_valid] = (
        xf[flat[flat_valid]] * dinv[flat[flat_valid]][:, None]
    ).T
    xs_all16 = xs_all.astype(bf16)

    per_core = []
    for c in range(n_cores):
        idx_calls = [np.zeros((runs, call_slots[b]), np.int64)
                     for b in range(nbanks)]
        tgtl_plane = np.full((P, k_cols), SENTINEL, np.float32)
        v_plane = np.zeros((P, k_cols), np.float32)
        for tau in range(tiles_per_core):
            gt = c * tiles_per_core + tau
            run, t7 = divmod(tau, run_tiles)
            for b in range(nbanks):
                g = gt * nbanks + b
                s0, s1 = starts[g], ends[g]
                n_g = s1 - s0
                assert n_g <= caps[b]
                sl = slice(s0, s1)
                srcs = all_src[sl] - bank_starts[b]
                tls = local_of[all_tgt[sl]]
                vs = all_v[sl]
                # slot s -> k-tile q=s//P, partition p=s%P
                s_arr = np.arange(n_g)
                qq = s_arr // P
                pp = s_arr % P
                cols = tau * k_tot + koff[b] + qq
                tgtl_plane[pp, cols] = tls
                v_plane[pp, cols] = vs
                idx_calls[b][run, t7 * caps[b] + s_arr] = srcs
        # wrap idx: element i of a call -> [i%16, i//16]; calls laid out
        # run-major, bank-minor; tiled over 128 partitions
        blocks = []
        for run in range(runs):
            for b in range(nbanks):
                iw = idx_calls[b][run].reshape(call_cols[b], 16)
                blocks.append(np.ascontiguousarray(iw.T))
        iw_all = np.concatenate(blocks, axis=1)
        idx_sb = np.tile(iw_all, (8, 1)).astype(np.int16)
        xs_c = np.ascontiguousarray(
            xs_all16[:, c * tiles_per_core * P:(c + 1) * tiles_per_core * P]
        )
        per_core.append(
            dict(idx_plane=idx_sb,
                 tgtl_plane=tgtl_plane,
                 v_plane=v_plane,
                 xs_plane=xs_c)
        )
    return per_core, node_order


def make_in_maps(x, weight, bias, per_core, cfg):
    bf16 = _bf16()
    n_pad = cfg["n_cores"] * cfg["tiles_per_core"] * P
    xr = np.zeros((n_pad, D), np.float32)
    xr[:N_NODES] = np.asarray(x, np.float32)
    xr16 = np.ascontiguousarray(xr.astype(bf16))
    w16 = np.ascontiguousarray(np.asarray(weight, np.float32).astype(bf16))
    iota = np.tile(np.arange(P, dtype=np.float32), (P, 1)).astype(bf16)
    bias_col = np.asarray(bias, np.float32).reshape(D, 1)
    in_maps = []
    for c in range(cfg["n_cores"]):
        in_maps.append(dict(
            xR=xr16,
            idx_plane=per_core[c]["idx_plane"],
            tgtl_plane=per_core[c]["tgtl_plane"],
            v_plane=per_core[c]["v_plane"],
            xs_plane=per_core[c]["xs_plane"],
            iota=iota,
            w=w16,
            bias_col=bias_col,
        ))
    return in_maps


# ----------------------------------------------------------------------------
# Kernel builder
# ----------------------------------------------------------------------------

def build(cfg, repeat=1):
    import concourse.bass as bass
    import concourse.mybir as mybir
    import concourse.tile as tile
    import concourse.bacc as bacc

    nbanks, caps, k_tot, koff, bank_starts = _derived(cfg)
    n_cores = cfg["n_cores"]
    tiles_per_core = cfg["tiles_per_core"]
    run_tiles = cfg["run_tiles"]
    kbs = cfg["kbs"]
    n_pad = n_cores * tiles_per_core * P
    runs = tiles_per_core // run_tiles
    call_slots = [run_tiles * caps[b] for b in range(nbanks)]
    call_cols = [cs // 16 for cs in call_slots]
    cols_per_run = sum(call_cols)
    k_cols = tiles_per_core * k_tot
    f32 = mybir.dt.float32
    bf16 = mybir.dt.bfloat16

    # chunk kk -> (bank, k-tile within bank)
    chunk_of = []
    for b in range(nbanks):
        for j in range(kbs[b]):
            chunk_of.append((b, j))
    assert len(chunk_of) == k_tot

    nc = bacc.Bacc("TRN2", target_bir_lowering=False, debug=False,
                   num_devices=n_cores, num_swdge_queues=4)
    x_d = nc.declare_dram_parameter("xR", [n_pad, D], bf16, isOutput=False)
    idx_d = nc.declare_dram_parameter(
        "idx_plane", [P, runs * cols_per_run], mybir.dt.int16,
        isOutput=False)
    tgtl_d = nc.declare_dram_parameter("tgtl_plane", [P, k_cols], f32,
                                       isOutput=False)
    v_d = nc.declare_dram_parameter("v_plane", [P, k_cols], f32,
                                    isOutput=False)
    xs_d = nc.declare_dram_parameter("xs_plane", [P, tiles_per_core * P],
                                     bf16, isOutput=False)
    iota_d = nc.declare_dram_parameter("iota", [P, P], bf16, isOutput=False)
    w_d = nc.declare_dram_parameter("w", [D, D], bf16, isOutput=False)
    bias_d = nc.declare_dram_parameter("bias_col", [D, 1], f32, isOutput=False)
    # transposed output: column block tau holds out[tile tau].T ([dout, t])
    out_d = nc.declare_dram_parameter("out", [P, tiles_per_core * P], bf16,
                                      isOutput=True)

    eq = mybir.AluOpType.is_equal
    mult = mybir.AluOpType.mult

    with tile.TileContext(nc) as tc:
        with (
            tc.tile_pool(name="const", bufs=1) as cpool,
            tc.tile_pool(name="gath", bufs=2) as gpool,
            tc.tile_pool(name="xsp", bufs=2) as xpool,
            tc.tile_pool(name="mt", bufs=4) as mpool,
            tc.tile_pool(name="agg", bufs=4) as apool,
            tc.tile_pool(name="outp", bufs=2) as opool,
            tc.tile_pool(name="psum", bufs=2, space="PSUM") as ppool,
            tc.tile_pool(name="psum2", bufs=2, space="PSUM") as p2pool,
        ):
            idx_sb = cpool.tile([P, runs * cols_per_run], mybir.dt.int16)
            tgtl_sb = cpool.tile([P, k_cols], f32)
            v_sb = cpool.tile([P, k_cols], f32)
            iota_sb = cpool.tile([P, P], bf16)
            w_sb = cpool.tile([D, D], bf16)
            bias_sb = cpool.tile([D, 1], f32)
            nc.sync.dma_start(out=idx_sb[:], in_=idx_d[:])
            nc.sync.dma_start(out=tgtl_sb[:], in_=tgtl_d[:])
            nc.sync.dma_start(out=v_sb[:], in_=v_d[:])
            nc.sync.dma_start(out=iota_sb[:], in_=iota_d[:])
            nc.sync.dma_start(out=w_sb[:], in_=w_d[:])
            nc.sync.dma_start(out=bias_sb[:], in_=bias_d[:])

            def body(_iv=None):
                for run in range(runs):
                    gts = []
                    col0 = run * cols_per_run
                    for b in range(nbanks):
                        g = gpool.tile([P, call_slots[b] // P, D], bf16,
                                       tag=f"g{b}")
                        lo = int(bank_starts[b])
                        hi = int(bank_starts[b + 1])
                        nc.gpsimd.dma_gather(
                            out_ap=g[:],
                            in_ap=x_d[lo:hi, :],
                            idxs_ap=idx_sb[:, col0:col0 + call_cols[b]],
                            num_idxs=call_slots[b],
                            num_idxs_reg=call_slots[b],
                            elem_size=D,
                            single_packet=False,
                            queue_num=b,
                        )
                        col0 += call_cols[b]
                        gts.append(g)
                    xs_run = xpool.tile([P, run_tiles * P], bf16, tag="xs")
                    t0 = run * run_tiles
                    nc.sync.dma_start(
                        out=xs_run[:],
                        in_=xs_d[:, t0 * P:(t0 + run_tiles) * P])
                    o_run = opool.tile([P, run_tiles, P], bf16, tag="o_run")
                    for t7 in range(run_tiles):
                        tau = run * run_tiles + t7
                        mbig = mpool.tile([P, k_tot, P], bf16, tag="mbig")
                        for kk in range(k_tot):
                            col = tau * k_tot + kk
                            nc.vector.tensor_scalar(
                                out=mbig[:, kk, :],
                                in0=iota_sb[:],
                                scalar1=tgtl_sb[:, col:col + 1],
                                scalar2=v_sb[:, col:col + 1],
                                op0=eq, op1=mult)
                        # psum1[din, t] = sum_slots x[src, din] * v one-hot
                        psum1 = ppool.tile([P, P], f32, tag="psum1")
                        for kk in range(k_tot):
                            b, j = chunk_of[kk]
                            hg = gts[b][:, t7 * kbs[b] + j, :]
                            nc.tensor.matmul(
                                psum1[:], lhsT=hg, rhs=mbig[:, kk, :],
                                start=(kk == 0), stop=(kk == k_tot - 1))
                        agg = apool.tile([P, P], bf16, tag="agg")
                        nc.scalar.copy(out=agg[:], in_=psum1[:])
                        # psum2[dout, t] = W^T @ (agg + xs_tile)
                        psum2 = p2pool.tile([P, P], f32, tag="psum2")
                        nc.tensor.matmul(psum2[:], lhsT=w_sb[:], rhs=agg[:],
                                         start=True, stop=False)
                        nc.tensor.matmul(
                            psum2[:], lhsT=w_sb[:],
                            rhs=xs_run[:, t7 * P:(t7 + 1) * P],
                            start=False, stop=True)
                        # bias is per-partition in this layout -> fold into
                        # the PSUM->SBUF copy on the scalar engine
                        nc.scalar.activation(
                            o_run[:, t7, :], psum2[:],
                            func=mybir.ActivationFunctionType.Identity,
                            bias=bias_sb[:, 0:1], scale=1.0)
                    nc.sync.dma_start(
                        out=out_d[:, t0 * P:(t0 + run_tiles) * P],
                        in_=o_run[:])

            if repeat == 1:
                body()
            else:
                with tc.For_i(0, repeat, 1) as iv:
                    body(iv)
    nc.compile()
    return nc


_CACHE = {}


def kernel(x, edge_index, edge_weight, weight, bias):
    from concourse.bass_utils import run_bass_kernel_spmd

    x = np.asarray(x, np.float32)
    edge_index = np.asarray(edge_index, np.int64)
    edge_weight = np.asarray(edge_weight, np.float32)
    weight = np.asarray(weight, np.float32)
    bias = np.asarray(bias, np.float32)

    cfg = _default_cfg()
    per_core, node_order = preprocess(x, edge_index, edge_weight, cfg)

    if "nc" not in _CACHE:
        _CACHE["nc"] = build(cfg)
    nc = _CACHE["nc"]

    in_maps = make_in_maps(x, weight, bias, per_core, cfg)
    res = run_bass_kernel_spmd(nc, in_maps, core_ids=list(range(cfg["n_cores"])))
    rows = np.concatenate(
        [np.ascontiguousarray(res.results[c]["out"].T).astype(np.float32)
         for c in range(cfg["n_cores"])], axis=0)
    out = np.zeros((N_NODES, D), np.float32)
    flat = node_order.reshape(-1)
    valid = flat < N_NODES
    out[flat[valid]] = rows[valid]
    return out


# revision 13
# speedup vs baseline: 1847868831.3845x; 1847868831.3845x over previous
"""DirectedGCNConv on 8 Trainium2 NeuronCores.

Math (reference):
    h    = x @ W
    agg  = segment_sum(w_e * h[src_e], tgt_e)
    deg  = segment_sum(w_e, tgt_e);  dinv = rsqrt(max(deg, 1))
    out  = (h + agg) * dinv + bias

Since segment-sum commutes with the feature matmul and dinv is a
per-target row scale:
    out[t] = ((dinv[t] x[t] + sum_e w_e dinv[t] x[src_e]) @ W) + bias
so dinv folds into per-edge values computed on the host, and the device
gathers raw x rows per edge (dma_gather), scatter-sums them per
128-target tile via one-hot matmuls into PSUM, applies the 128x128
weight matmul, and adds bias.  No h precompute: the weight matmul is
applied AFTER aggregation, once per target tile, so nothing but the
gathered rows ever moves through HBM.

The self term dinv[t]*x[t] is not gathered: the host prepares a
dinv-scaled, node-ordered, transposed copy of x (xs), which streams in
with one sequential DMA per run and enters psum2 as a second
accumulating matmul against W.

Sharding: node-parallel by target.  The 100k nodes are packed into
784 tiles of 128, bin-packed so that, per tile, the incoming-edge count
from source-index bank b fits in kbs[b] K-tiles of 128 slots (banks
29184/29184/29184/12800 rows at 2/2/2/1 K-tiles = 7 K-chunks per tile,
896 gather slots for a mean real edge load of 816).  Each core owns 98
tiles and processes every edge targeting its nodes.  x and W are
replicated; no collectives are needed.

The gather uses the Q7 dma_gather instruction (int16 indices, hence
banks of < 32768 rows).  All matmul/gather data is bf16; accumulation
stays fp32 in PSUM.  One-hot scatter matrices are built in-loop with
one fused tensor_scalar (is_equal then mult with fp32 per-partition
scalars) per K-chunk on the vector engine.
"""

import sys

import numpy as np

for _p in ("/opt/trn_rl_repo", "/root/.axon_site/_ro/trn_rl_repo"):
    if _p not in sys.path:
        sys.path.append(_p)

N_NODES = 100000
N_EDGES = 640000
D = 128
N_CORES = 8

P = 128                      # partitions / tile node count
SENTINEL = 200.0             # tgtl value for phantom slots (bf16-exact)


def _default_cfg():
    return dict(
        n_nodes=N_NODES,
        n_cores=N_CORES,
        tiles_per_core=98,
        run_tiles=14,                  # tiles per gather "run"
        bank_sizes=(29184, 29184, 29184, 12800),   # int16-safe, sum=n_pad
        kbs=(2, 2, 2, 1),              # K-tiles per (tile, bank)
    )


def _derived(cfg):
    kbs = cfg["kbs"]
    bank_sizes = cfg["bank_sizes"]
    nbanks = len(kbs)
    caps = [kb * P for kb in kbs]
    k_tot = sum(kbs)
    koff = np.concatenate([[0], np.cumsum(kbs)[:-1]]).astype(int)
    bank_starts = np.concatenate([[0], np.cumsum(bank_sizes)]).astype(int)
    return nbanks, caps, k_tot, koff, bank_starts


def _bf16():
    import ml_dtypes
    return ml_dtypes.bfloat16


# ----------------------------------------------------------------------------
# Host-side preprocessing
# ----------------------------------------------------------------------------

def _pack_nodes(tgt, cfg):
    """Assign each node to a (tile, slot) so that per-(tile, bank)
    edge counts are <= caps[b].  Returns node_order [n_tiles, P] holding
    node ids (or >= n_nodes for phantom slots)."""
    nbanks, caps, _, _, bank_starts = _derived(cfg)
    n_tiles = cfg["n_cores"] * cfg["tiles_per_core"]
    n_slots = n_tiles * P
    caps = np.asarray(caps)

    src = cfg["_src"]
    bank_of_src = (np.searchsorted(bank_starts, src, "right") - 1).astype(
        np.int64)
    # per-node, per-bank demand (incoming edges by source bank)
    d = np.zeros((n_slots, nbanks), np.int64)
    np.add.at(d, (tgt, bank_of_src), 1)

    total = d.sum(1)
    order = np.argsort(-total, kind="stable")
    # serpentine init: exactly P nodes (incl. phantoms) per tile
    assign = np.empty(n_slots, np.int64)
    row = np.arange(n_slots) // n_tiles
    pos = np.arange(n_slots) % n_tiles
    assign[order] = np.where(row % 2 == 0, pos, n_tiles - 1 - pos)
    loads = np.zeros((n_tiles, nbanks), np.int64)
    np.add.at(loads, assign, d)
    counts = np.bincount(assign, minlength=n_tiles)
    assert (counts == P).all()

    # repair: swap nodes out of overloaded (tile, bank)
    members = [list(np.where(assign == t)[0]) for t in range(n_tiles)]
    for _ in range(400):
        over = np.argwhere(loads > caps[None, :])
        if len(over) == 0:
            break
        for t, b in over:
            while loads[t, b] > caps[b]:
                mem = members[t]
                n_out = mem[int(np.argmax(d[mem, b]))]
                # candidate destination tiles, lightest in bank b
                cand = np.argsort(loads[:, b] - caps[b])
                done = False
                for u in cand[:128]:
                    if u == t:
                        continue
                    memu = members[u]
                    n_in = memu[int(np.argmin(d[memu, b]))]
                    new_u = loads[u] - d[n_in] + d[n_out]
                    new_t = loads[t] - d[n_out] + d[n_in]
                    if (new_u <= caps).all() and (new_t[b] < loads[t, b]):
                        members[t].remove(n_out)
                        members[u].remove(n_in)
                        members[t].append(n_in)
                        members[u].append(n_out)
                        loads[t] = new_t
                        loads[u] = new_u
                        assign[n_out] = u
                        assign[n_in] = t
                        done = True
                        break
                if not done:
                    raise RuntimeError("packing repair failed")
    assert (loads <= caps[None, :]).all(), "node packing failed"

    node_order = np.empty((n_tiles, P), np.int64)
    for t in range(n_tiles):
        node_order[t] = sorted(members[t])
    return node_order


def preprocess(x, edge_index, edge_weight, cfg):
    """Build per-core device arrays.  Returns (per_core_inputs, node_order)."""
    bf16 = _bf16()
    nbanks, caps, k_tot, koff, bank_starts = _derived(cfg)
    n_nodes = cfg["n_nodes"]
    n_cores = cfg["n_cores"]
    tiles_per_core = cfg["tiles_per_core"]
    run_tiles = cfg["run_tiles"]
    n_tiles = n_cores * tiles_per_core
    runs = tiles_per_core // run_tiles
    call_slots = [run_tiles * caps[b] for b in range(nbanks)]
    # idx-plane column offset of each call (run-major, bank-minor), in
    # 16-element wraps
    call_cols = [cs // 16 for cs in call_slots]
    cols_per_run = sum(call_cols)
    k_cols = tiles_per_core * k_tot           # tgtl/v plane columns per core

    src = np.asarray(edge_index[0], np.int64)
    tgt = np.asarray(edge_index[1], np.int64)
    w = np.asarray(edge_weight, np.float32)
    cfg = dict(cfg)
    cfg["_src"] = src

    node_order = _pack_nodes(tgt, cfg)

    # degree normalization, folded into the per-edge values
    deg = np.zeros(n_nodes, np.float64)
    np.add.at(deg, tgt, w.astype(np.float64))
    dinv = (1.0 / np.sqrt(np.maximum(deg, 1.0))).astype(np.float32)

    # node -> (tile, local slot)
    tile_of = np.empty(n_tiles * P, np.int64)
    local_of = np.empty(n_tiles * P, np.int64)
    flat = node_order.reshape(-1)
    tile_of[flat] = np.arange(n_tiles * P) // P
    local_of[flat] = np.arange(n_tiles * P) % P

    # per-edge values: v = w_e * dinv[tgt]  (self term handled via xs)
    all_src = src
    all_tgt = tgt
    all_v = w * dinv[tgt]

    e_tile = tile_of[all_tgt]
    e_bank = (np.searchsorted(bank_starts, all_src, "right") - 1).astype(
        np.int64)
    order = np.lexsort((e_bank, e_tile))
    all_src, all_tgt, all_v, e_tile, e_bank = (
        a[order] for a in (all_src, all_tgt, all_v, e_tile, e_bank)
    )
    group_key = e_tile * nbanks + e_bank
    starts = np.searchsorted(group_key, np.arange(n_tiles * nbanks))
    ends = np.searchsorted(group_key, np.arange(n_tiles * nbanks) + 1)

    # dinv-scaled x^T in node order: xs[:, gt*P + p] = x[node].T * dinv[node]
    xf = np.asarray(x, np.float32)
    flat_valid = flat < n_nodes
    xs_all = np.zeros((D, n_tiles * P), np.float32)
    xs_all[:, flat_valid] = (
        xf[flat[flat_valid]] * dinv[flat[flat_valid]][:, None]
    ).T
    xs_all16 = xs_all.astype(bf16)

    per_core = []
    for c in range(n_cores):
        idx_calls = [np.zeros((runs, call_slots[b]), np.int64)
                     for b in range(nbanks)]
        tgtl_plane = np.full((P, k_cols), SENTINEL, np.float32)
        v_plane = np.zeros((P, k_cols), np.float32)
        for tau in range(tiles_per_core):
            gt = c * tiles_per_core + tau
            run, t7 = divmod(tau, run_tiles)
            for b in range(nbanks):
                g = gt * nbanks + b
                s0, s1 = starts[g], ends[g]
                n_g = s1 - s0
                assert n_g <= caps[b]
                sl = slice(s0, s1)
                srcs = all_src[sl] - bank_starts[b]
                tls = local_of[all_tgt[sl]]
                vs = all_v[sl]
                # slot s -> k-tile q=s//P, partition p=s%P
                s_arr = np.arange(n_g)
                qq = s_arr // P
                pp = s_arr % P
                cols = tau * k_tot + koff[b] + qq
                tgtl_plane[pp, cols] = tls
                v_plane[pp, cols] = vs
                idx_calls[b][run, t7 * caps[b] + s_arr] = srcs
        # wrap idx: element i of a call -> [i%16, i//16]; calls laid out
        # run-major, bank-minor; tiled over 128 partitions
        blocks = []
        for run in range(runs):
            for b in range(nbanks):
                iw = idx_calls[b][run].reshape(call_cols[b], 16)
                blocks.append(np.ascontiguousarray(iw.T))
        iw_all = np.concatenate(blocks, axis=1)
        idx_sb = np.tile(iw_all, (8, 1)).astype(np.int16)
        xs_c = np.ascontiguousarray(
            xs_all16[:, c * tiles_per_core * P:(c + 1) * tiles_per_core * P]
        )
        per_core.append(
            dict(idx_plane=idx_sb,
                 tgtl_plane=tgtl_plane,
                 v_plane=v_plane,
                 xs_plane=xs_c)
        )
    return per_core, node_order


def make_in_maps(x, weight, bias, per_core, cfg):
    bf16 = _bf16()
    n_pad = cfg["n_cores"] * cfg["tiles_per_core"] * P
    xr = np.zeros((n_pad, D), np.float32)
    xr[:N_NODES] = np.asarray(x, np.float32)
    xr16 = np.ascontiguousarray(xr.astype(bf16))
    w16 = np.ascontiguousarray(np.asarray(weight, np.float32).astype(bf16))
    iota = np.tile(np.arange(P, dtype=np.float32), (P, 1)).astype(bf16)
    bias_col = np.asarray(bias, np.float32).reshape(D, 1)
    in_maps = []
    for c in range(cfg["n_cores"]):
        in_maps.append(dict(
            xR=xr16,
            idx_plane=per_core[c]["idx_plane"],
            tgtl_plane=per_core[c]["tgtl_plane"],
            v_plane=per_core[c]["v_plane"],
            xs_plane=per_core[c]["xs_plane"],
            iota=iota,
            w=w16,
            bias_col=bias_col,
        ))
    return in_maps


# ----------------------------------------------------------------------------
# Kernel builder
# ----------------------------------------------------------------------------

def build(cfg, repeat=1, unroll=False):
    import concourse.bass as bass
    import concourse.mybir as mybir
    import concourse.tile as tile
    import concourse.bacc as bacc

    nbanks, caps, k_tot, koff, bank_starts = _derived(cfg)
    n_cores = cfg["n_cores"]
    tiles_per_core = cfg["tiles_per_core"]
    run_tiles = cfg["run_tiles"]
    kbs = cfg["kbs"]
    n_pad = n_cores * tiles_per_core * P
    runs = tiles_per_core // run_tiles
    call_slots = [run_tiles * caps[b] for b in range(nbanks)]
    call_cols = [cs // 16 for cs in call_slots]
    cols_per_run = sum(call_cols)
    k_cols = tiles_per_core * k_tot
    f32 = mybir.dt.float32
    bf16 = mybir.dt.bfloat16

    # chunk kk -> (bank, k-tile within bank)
    chunk_of = []
    for b in range(nbanks):
        for j in range(kbs[b]):
            chunk_of.append((b, j))
    assert len(chunk_of) == k_tot

    nc = bacc.Bacc("TRN2", target_bir_lowering=False, debug=False,
                   num_devices=n_cores, num_swdge_queues=4)
    x_d = nc.declare_dram_parameter("xR", [n_pad, D], bf16, isOutput=False)
    idx_d = nc.declare_dram_parameter(
        "idx_plane", [P, runs * cols_per_run], mybir.dt.int16,
        isOutput=False)
    tgtl_d = nc.declare_dram_parameter("tgtl_plane", [P, k_cols], f32,
                                       isOutput=False)
    v_d = nc.declare_dram_parameter("v_plane", [P, k_cols], f32,
                                    isOutput=False)
    xs_d = nc.declare_dram_parameter("xs_plane", [P, tiles_per_core * P],
                                     bf16, isOutput=False)
    iota_d = nc.declare_dram_parameter("iota", [P, P], bf16, isOutput=False)
    w_d = nc.declare_dram_parameter("w", [D, D], bf16, isOutput=False)
    bias_d = nc.declare_dram_parameter("bias_col", [D, 1], f32, isOutput=False)
    # transposed output: column block tau holds out[tile tau].T ([dout, t])
    out_d = nc.declare_dram_parameter("out", [P, tiles_per_core * P], bf16,
                                      isOutput=True)

    eq = mybir.AluOpType.is_equal
    mult = mybir.AluOpType.mult

    with tile.TileContext(nc) as tc:
        with (
            tc.tile_pool(name="const", bufs=1) as cpool,
            tc.tile_pool(name="gath", bufs=3) as gpool,
            tc.tile_pool(name="xsp", bufs=2) as xpool,
            tc.tile_pool(name="mt", bufs=4) as mpool,
            tc.tile_pool(name="agg", bufs=4) as apool,
            tc.tile_pool(name="outp", bufs=2) as opool,
            tc.tile_pool(name="psum", bufs=4, space="PSUM") as ppool,
            tc.tile_pool(name="psum2", bufs=4, space="PSUM") as p2pool,
        ):
            idx_sb = cpool.tile([P, runs * cols_per_run], mybir.dt.int16)
            tgtl_sb = cpool.tile([P, k_cols], f32)
            v_sb = cpool.tile([P, k_cols], f32)
            iota_sb = cpool.tile([P, P], bf16)
            w_sb = cpool.tile([D, D], bf16)
            bias_sb = cpool.tile([D, 1], f32)
            nc.sync.dma_start(out=idx_sb[:], in_=idx_d[:])
            nc.sync.dma_start(out=tgtl_sb[:], in_=tgtl_d[:])
            nc.sync.dma_start(out=v_sb[:], in_=v_d[:])
            nc.sync.dma_start(out=iota_sb[:], in_=iota_d[:])
            nc.sync.dma_start(out=w_sb[:], in_=w_d[:])
            nc.sync.dma_start(out=bias_sb[:], in_=bias_d[:])

            variant = cfg.get("_variant", "full")

            def body(_iv=None):
                for run in range(runs):
                    gts = []
                    col0 = run * cols_per_run
                    for b in range(nbanks):
                        if variant == "nogather":
                            gts.append(None)
                            continue
                        g = gpool.tile([P, call_slots[b] // P, D], bf16,
                                       tag=f"g{b}")
                        lo = int(bank_starts[b])
                        hi = int(bank_starts[b + 1])
                        nc.gpsimd.dma_gather(
                            out_ap=g[:],
                            in_ap=x_d[lo:hi, :],
                            idxs_ap=idx_sb[:, col0:col0 + call_cols[b]],
                            num_idxs=call_slots[b],
                            num_idxs_reg=call_slots[b],
                            elem_size=D,
                            single_packet=False,
                            queue_num=b,
                        )
                        col0 += call_cols[b]
                        gts.append(g)
                    if variant == "gatheronly":
                        continue
                    xs_run = xpool.tile([P, run_tiles * P], bf16, tag="xs")
                    t0 = run * run_tiles
                    nc.sync.dma_start(
                        out=xs_run[:],
                        in_=xs_d[:, t0 * P:(t0 + run_tiles) * P])
                    o_run = opool.tile([P, run_tiles, P], bf16, tag="o_run")

                    # Stage 2 of the software pipeline: W matmuls + bias
                    # activation for tile t7, issued one tile late so the
                    # PSUM->SBUF copy they depend on never stalls the PE /
                    # ACT queue heads.
                    def finish_tile(t7, agg):
                        # psum2[dout, t] = W^T @ (agg + xs_tile)
                        psum2 = p2pool.tile([P, P], f32, tag="psum2")
                        nc.tensor.matmul(psum2[:], lhsT=w_sb[:], rhs=agg[:],
                                         start=True, stop=False)
                        nc.tensor.matmul(
                            psum2[:], lhsT=w_sb[:],
                            rhs=xs_run[:, t7 * P:(t7 + 1) * P],
                            start=False, stop=True)
                        # bias is per-partition in this layout -> fold into
                        # the PSUM->SBUF copy on the scalar engine
                        nc.scalar.activation(
                            o_run[:, t7, :], psum2[:],
                            func=mybir.ActivationFunctionType.Identity,
                            bias=bias_sb[:, 0:1], scale=1.0)

                    pending = None
                    for t7 in range(run_tiles):
                        tau = run * run_tiles + t7
                        mbig = mpool.tile([P, k_tot, P], bf16, tag="mbig")
                        for kk in range(k_tot):
                            col = tau * k_tot + kk
                            nc.vector.tensor_scalar(
                                out=mbig[:, kk, :],
                                in0=iota_sb[:],
                                scalar1=tgtl_sb[:, col:col + 1],
                                scalar2=v_sb[:, col:col + 1],
                                op0=eq, op1=mult)
                        # psum1[din, t] = sum_slots x[src, din] * v one-hot
                        psum1 = ppool.tile([P, P], f32, tag="psum1")
                        for kk in range(k_tot):
                            b, j = chunk_of[kk]
                            if variant == "nogather":
                                hg = iota_sb[:]
                            else:
                                hg = gts[b][:, t7 * kbs[b] + j, :]
                            nc.tensor.matmul(
                                psum1[:], lhsT=hg, rhs=mbig[:, kk, :],
                                start=(kk == 0), stop=(kk == k_tot - 1))
                        agg = apool.tile([P, P], bf16, tag="agg")
                        nc.scalar.copy(out=agg[:], in_=psum1[:])
                        if pending is not None:
                            finish_tile(*pending)
                        pending = (t7, agg)
                    finish_tile(*pending)
                    nc.sync.dma_start(
                        out=out_d[:, t0 * P:(t0 + run_tiles) * P],
                        in_=o_run[:])

            if repeat == 1:
                body()
            elif unroll:
                for _ in range(repeat):
                    body()
            else:
                with tc.For_i(0, repeat, 1) as iv:
                    body(iv)
    nc.compile()
    return nc


_CACHE = {}


def kernel(x, edge_index, edge_weight, weight, bias):
    from concourse.bass_utils import run_bass_kernel_spmd

    x = np.asarray(x, np.float32)
    edge_index = np.asarray(edge_index, np.int64)
    edge_weight = np.asarray(edge_weight, np.float32)
    weight = np.asarray(weight, np.float32)
    bias = np.asarray(bias, np.float32)

    cfg = _default_cfg()
    per_core, node_order = preprocess(x, edge_index, edge_weight, cfg)

    if "nc" not in _CACHE:
        _CACHE["nc"] = build(cfg)
    nc = _CACHE["nc"]

    in_maps = make_in_maps(x, weight, bias, per_core, cfg)
    res = run_bass_kernel_spmd(nc, in_maps, core_ids=list(range(cfg["n_cores"])))
    rows = np.concatenate(
        [np.ascontiguousarray(res.results[c]["out"].T).astype(np.float32)
         for c in range(cfg["n_cores"])], axis=0)
    out = np.zeros((N_NODES, D), np.float32)
    flat = node_order.reshape(-1)
    valid = flat < N_NODES
    out[flat[valid]] = rows[valid]
    return out
